# revision 26
# baseline (speedup 1.0000x reference)
"""Bass/Trainium2 kernel for nn_BiMambaBlock (Mamba2 block), 8-core SPMD.

Sharding: 8 shards = (batch b in 0..3) x (sequence half s in 0,1); each core
processes 1024 tokens of one batch end-to-end in fp16 on-chip compute
(fp32 accumulation), using the chunked-SSD formulation (chunk Q=128).
Cross-core dependency: SSM state at the sequence midpoint, passed via a
pairwise AllGather; the receiving core applies a low-rank correction
(Delta-Y = C~global @ h_init) before gating/norm/out_proj.

Decay matrices (exp(cs_j - cs_i) per head) are built on the PE via a
rank-33 matmul (ones/cs stationary against a [cs-flat; -head-selector]
moving matrix); partition-broadcasts of exp(cs) rows come from stride-0
DMAs, keeping the vector engine to one fused multiply per tile.
"""
import sys
import numpy as np

for p in ("/opt/trn_rl_repo", "/opt/trn_rl_repo/concourse"):
    if p not in sys.path:
        sys.path.insert(0, p)

import concourse.bass as bass
import concourse.bacc as bacc
import concourse.tile as tile
from concourse import mybir
from concourse.bass_utils import run_bass_kernel_spmd

F32 = mybir.dt.float32
F16 = mybir.dt.float16
AX = mybir.AluOpType
AF = mybir.ActivationFunctionType

D_MODEL = 1024
D_STATE = 64
NHEADS = 32
HEADDIM = 64
D_INNER = 2048
B_SZ = 4
L = 2048
S = 1024          # per-core segment
T = 1152          # padded token window (3 halo + 1024 + 125 pad)
Q = 128           # SSD chunk
NCHUNK = S // Q
NCORES = 8
DIP_EXT = 4384    # z(2048) + x(2048) + [B|C](128) + [C|B](128) + dt(32)

_CACHE = {}


def _rep_inner(sl, rep):
    """[P, n] slice -> [P, n, rep(stride0)] broadcast AP."""
    return bass.AP(tensor=sl.tensor, offset=sl.offset,
                   ap=[sl.ap[0], sl.ap[1], [0, rep]])


def _rep_outer(sl, rep):
    """[P, n] slice -> [P, rep(stride0), n] broadcast AP."""
    return bass.AP(tensor=sl.tensor, offset=sl.offset,
                   ap=[sl.ap[0], [0, rep], sl.ap[1]])


def _col_bcast(col, n):
    """[P, 1] slice -> [P, n(stride0)] broadcast AP."""
    return bass.AP(tensor=col.tensor, offset=col.offset,
                   ap=[col.ap[0], [0, n]])


def _part_bcast(sl, nparts):
    """[1, n] slice -> [1, nparts(stride0), n] broadcast AP (DMA source).

    Partition dim must have nonzero step, so the repeat lives in a
    stride-0 free dim on the single source partition.
    """
    return bass.AP(tensor=sl.tensor, offset=sl.offset,
                   ap=[[1, 1], [0, nparts]] + list(sl.ap[1:]))


def _flat_rows(sl, nrows, ncols):
    """[nrows, ncols] tile viewed as [1, nrows, ncols] free dims (DMA)."""
    # destination AP for a row-major flatten of a multi-partition tile
    return bass.AP(tensor=sl.tensor, offset=sl.offset,
                   ap=[sl.ap[0], [ncols, nrows], [1, ncols]])


def build_program():
    import os
    _PH = int(os.environ.get("KPHASES", "4"))
    _REP = int(os.environ.get("KREPEAT", "1"))
    nc = bacc.Bacc("TRN2", target_bir_lowering=False, debug=False,
                   num_devices=NCORES)
    nc.cache_partition_id()
    _build_body(nc, _PH, _REP)
    nc.compile()
    return nc


def _build_body(nc, _PH, _REP=1):
    uT16 = nc.declare_dram_parameter("uT16", [D_MODEL, T], F16, isOutput=False)
    Win = nc.declare_dram_parameter("Win", [D_MODEL, DIP_EXT], F16, isOutput=False)
    Wout = nc.declare_dram_parameter("Wout", [D_INNER, D_MODEL], F16, isOutput=False)
    convw = nc.declare_dram_parameter("convw", [18 * 128, 4], F32, isOutput=False)
    convb = nc.declare_dram_parameter("convb", [18 * 128, 1], F32, isOutput=False)
    dtb = nc.declare_dram_parameter("dtb", [NHEADS, 1], F32, isOutput=False)
    Acol = nc.declare_dram_parameter("Acol", [NHEADS, 1], F32, isOutput=False)
    Dexp = nc.declare_dram_parameter("Dexp", [D_INNER, 1], F32, isOutput=False)
    nwexp = nc.declare_dram_parameter("nwexp", [D_INNER, 1], F32, isOutput=False)
    ones_c = nc.declare_dram_parameter("ones_c", [128, 128], F16, isOutput=False)
    id16_c = nc.declare_dram_parameter("id16_c", [128, 128], F16, isOutput=False)
    id32_c = nc.declare_dram_parameter("id32_c", [128, 128], F32, isOutput=False)
    triu_c = nc.declare_dram_parameter("triu_c", [128, 128], F16, isOutput=False)
    nghs_c = nc.declare_dram_parameter("nghs_c", [33, NHEADS * 128], F16,
                                       isOutput=False)
    mselp = nc.declare_dram_parameter("mselp", [D_STATE, 4], F32, isOutput=False)
    epsc = nc.declare_dram_parameter("epsc", [128, 1], F32, isOutput=False)
    outT = nc.declare_dram_parameter("outT", [D_MODEL, S], F32, isOutput=True)

    ctilg_d = nc.dram_tensor("ctilg_d", [D_STATE, NCHUNK * NHEADS * 128], F16)
    ccin = nc.dram_tensor("ccin", [D_STATE, D_INNER], F16)
    ccout = nc.dram_tensor("ccout", [NCORES, D_STATE, D_INNER], F16,
                           addr_space="Shared")

    for _rep in range(_REP):
        _build_once(nc, _PH, uT16, Win, Wout, convw, convb, dtb, Acol, Dexp,
                    nwexp, ones_c, id16_c, id32_c, triu_c, mselp, epsc, nghs_c,
                    outT, ctilg_d, ccin, ccout)


def _build_once(nc, _PH, uT16, Win, Wout, convw, convb, dtb, Acol, Dexp,
                nwexp, ones_c, id16_c, id32_c, triu_c, mselp, epsc, nghs_c,
                outT, ctilg_d, ccin, ccout):
    import os
    from contextlib import ExitStack
    with tile.TileContext(nc) as tc, ExitStack() as ctx:
        perm = ctx.enter_context(tc.tile_pool(name="perm", bufs=1))

        ones16 = perm.tile([128, 128], F16)
        nc.sync.dma_start(out=ones16, in_=ones_c[:, :])
        id16 = perm.tile([128, 128], F16)
        nc.sync.dma_start(out=id16, in_=id16_c[:, :])
        id32 = perm.tile([128, 128], F32)
        nc.sync.dma_start(out=id32, in_=id32_c[:, :])
        triu16 = perm.tile([128, 128], F16)
        nc.sync.dma_start(out=triu16, in_=triu_c[:, :])
        dtb_sb = perm.tile([NHEADS, 1], F32)
        nc.sync.dma_start(out=dtb_sb, in_=dtb[:, :])
        A_sb = perm.tile([NHEADS, 1], F32)
        nc.sync.dma_start(out=A_sb, in_=Acol[:, :])
        msel_sb = perm.tile([D_STATE, 4], F32)
        nc.sync.dma_start(out=msel_sb, in_=mselp[:, :])
        eps_sb = perm.tile([128, 1], F32)
        nc.sync.dma_start(out=eps_sb, in_=epsc[:, :])
        cw_sb = perm.tile([128, 18 * 4], F32)
        cb_sb = perm.tile([128, 18], F32)
        for t in range(18):
            nc.sync.dma_start(out=cw_sb[:, t * 4:(t + 1) * 4],
                              in_=convw[t * 128:(t + 1) * 128, :])
            nc.sync.dma_start(out=cb_sb[:, t:t + 1],
                              in_=convb[t * 128:(t + 1) * 128, :])
        D_sb = perm.tile([128, 16], F32)
        nw_sb = perm.tile([128, 16], F32)
        for t in range(16):
            nc.sync.dma_start(out=D_sb[:, t:t + 1],
                              in_=Dexp[t * 128:(t + 1) * 128, :])
            nc.sync.dma_start(out=nw_sb[:, t:t + 1],
                              in_=nwexp[t * 128:(t + 1) * 128, :])
        # rhs33: row0 = per-chunk cs-flat (DMA'd), rows 1..32 = -head-selector
        rhs33 = perm.tile([33, NHEADS * 128], F16)
        nc.sync.dma_start(out=rhs33, in_=nghs_c[:, :])
        stat33 = perm.tile([33, 128], F16)
        nc.vector.memset(stat33[32:33, :], 1.0)

        zs = [perm.tile([128, S], F16, tag=f"zs{t}", name=f"zs{t}")
              for t in range(16)]
        xcv = [perm.tile([128, S], F16, tag=f"xcv{t}", name=f"xcv{t}")
               for t in range(16)]
        bt64 = perm.tile([D_STATE, S], F16)   # B^T  (state-dim on partitions)
        ct64 = perm.tile([D_STATE, S], F16)   # C^T
        dtT = perm.tile([NHEADS, T], F32)

        # allocated early so its SBUF region does not alias W_in space ->
        # the scan can run while z matmuls still stream
        csTg = perm.tile([NHEADS, S], F32)

        # ================= P1: in_proj + conv =================
        # ct order: dt first (34), then xBC (16..33), then z (0..15) so the
        # scan + P2 chunks can start while z matmuls still run.
        with tc.tile_pool(name="win", bufs=1) as winp, \
             tc.tile_pool(name="ut", bufs=1) as utp, \
             tc.tile_pool(name="ippsum", bufs=2, space="PSUM") as ipp, \
             tc.tile_pool(name="cvtmp", bufs=2) as cvp:
            uT = [utp.tile([128, T], F16, tag=f"ut{k}", name=f"ut{k}")
                  for k in range(8)]
            for k in range(8):
                nc.sync.dma_start(out=uT[k],
                                  in_=uT16[k * 128:(k + 1) * 128, :])
            W_sb = [winp.tile([128, DIP_EXT], F16, tag=f"w{k}", name=f"w{k}")
                    for k in range(8)]
            for k in range(8):
                nc.sync.dma_start(out=W_sb[k], in_=Win[k * 128:(k + 1) * 128, :])

            ct_order = [34] + list(range(16, 34)) + list(range(0, 16))
            _CTMAX = int(os.environ.get('KCT', '35'))
            for ct in ct_order[:_CTMAX]:
                ncols = 128 if ct < 34 else 32
                c0 = ct * 128
                ip = ipp.tile([128, T], F32, tag="ip")
                for k in range(8):
                    st, sp = (k == 0), (k == 7)
                    lt = W_sb[k][:, c0:c0 + ncols]
                    nc.tensor.matmul(ip[0:ncols, 0:512], lt, uT[k][:, 0:512],
                                     start=st, stop=sp)
                    nc.tensor.matmul(ip[0:ncols, 512:1024], lt,
                                     uT[k][:, 512:1024], start=st, stop=sp)
                    nc.tensor.matmul(ip[0:ncols, 1024:1152], lt,
                                     uT[k][:, 1024:1152], start=st, stop=sp)
                if ct < 16:
                    nc.scalar.activation(zs[ct], ip[:, 3:3 + S], AF.Silu)
                elif ct < 34:
                    xi = ct - 16
                    rows = 128 if xi < 16 else 64
                    xbc = cvp.tile([128, T], F16, tag="xbc")
                    nc.scalar.copy(xbc[0:rows, :], ip[0:rows, :])
                    w4 = cw_sb[:, xi * 4:(xi + 1) * 4]
                    t1 = cvp.tile([128, S], F16, tag="cv1")
                    nc.vector.tensor_scalar_mul(t1[0:rows, :], xbc[0:rows, 0:S],
                                                w4[0:rows, 0:1])
                    t2 = cvp.tile([128, S], F16, tag="cv2")
                    nc.vector.scalar_tensor_tensor(
                        t2[0:rows, :], xbc[0:rows, 1:1 + S], w4[0:rows, 1:2],
                        t1[0:rows, :], op0=AX.mult, op1=AX.add)
                    t3 = cvp.tile([128, S], F16, tag="cv1")
                    nc.vector.scalar_tensor_tensor(
                        t3[0:rows, :], xbc[0:rows, 2:2 + S], w4[0:rows, 2:3],
                        t2[0:rows, :], op0=AX.mult, op1=AX.add)
                    t4 = cvp.tile([128, S], F16, tag="cv2")
                    nc.vector.scalar_tensor_tensor(
                        t4[0:rows, :], xbc[0:rows, 3:3 + S], w4[0:rows, 3:4],
                        t3[0:rows, :], op0=AX.mult, op1=AX.add)
                    if xi < 16:
                        nc.scalar.activation(xcv[xi], t4, AF.Silu,
                                             bias=cb_sb[:, xi:xi + 1])
                    elif xi == 16:
                        nc.scalar.activation(bt64, t4[0:64, :], AF.Silu,
                                             bias=cb_sb[0:64, xi:xi + 1])
                    else:
                        nc.scalar.activation(ct64, t4[0:64, :], AF.Silu,
                                             bias=cb_sb[0:64, xi:xi + 1])
                else:
                    # cumulative sum of dt*A over the segment (scan)
                    with tc.tile_pool(name="scanp", bufs=1) as scp:
                        dte = scp.tile([NHEADS, T], F32, tag="dte")
                        nc.scalar.activation(dte, ip[0:NHEADS, :], AF.Exp,
                                             bias=dtb_sb[:, 0:1])
                        nc.scalar.activation(dtT, dte, AF.Ln, bias=1.0)
                        aT = scp.tile([NHEADS, S], F32)
                        zrow = scp.tile([NHEADS, S], F32)
                        nc.vector.tensor_scalar_mul(aT, dtT[:, 3:3 + S],
                                                    A_sb[:, 0:1])
                        nc.vector.memset(zrow, 0.0)
                        nc.vector.tensor_tensor_scan(csTg, aT, zrow, 0.0,
                                                     op0=AX.add, op1=AX.add)

        if _CTMAX < 35 or _PH < 2:
            osb0 = perm.tile([128, S], F32)
            nc.scalar.copy(osb0[0:NHEADS, :], csTg)
            for dm in range(8):
                nc.sync.dma_start(out=outT[dm * 128:(dm + 1) * 128, :],
                                  in_=osb0)
            return

        # pool for tensors first written during/after P2 (reuses W_in space)
        perm2 = ctx.enter_context(tc.tile_pool(name="perm2", bufs=1))
        ysb = [perm2.tile([128, S], F16, tag=f"ysb{t}", name=f"ysb{t}")
               for t in range(16)]
        hfin = perm2.tile([D_STATE, D_INNER], F16)
        hinit = perm2.tile([D_STATE, D_INNER], F16)

        # ================= P2: SSD chunks =================
        with tc.tile_pool(name="trp", bufs=1, space="PSUM") as trp, \
             tc.tile_pool(name="xdtp", bufs=2, space="PSUM") as xdtp, \
             tc.tile_pool(name="dbp", bufs=1, space="PSUM") as dbp, \
             tc.tile_pool(name="spsp", bufs=1, space="PSUM") as spsp, \
             tc.tile_pool(name="ypsp", bufs=2, space="PSUM") as ypsp, \
             tc.tile_pool(name="ck", bufs=2) as ckp, \
             tc.tile_pool(name="eb", bufs=2) as ebp, \
             tc.tile_pool(name="rp", bufs=1) as rpp, \
             tc.tile_pool(name="hp", bufs=2) as hpool:
            hprev = None
            for c in range(NCHUNK):
                W = slice(c * Q, (c + 1) * Q)
                csl = ckp.tile([NHEADS, Q], F32, tag="csl")
                if c == 0:
                    nc.vector.tensor_copy(csl, csTg[:, W])
                else:
                    nc.vector.tensor_scalar(csl, csTg[:, W],
                                            csTg[:, c * Q - 1:c * Q], None,
                                            op0=AX.subtract)
                csT16 = ckp.tile([NHEADS, Q], F16, tag="cst16")
                nc.vector.tensor_copy(csT16, csl)
                # decay-delta matmul stationary: rows0:32=cs, row32=ones
                nc.vector.tensor_copy(stat33[0:32, :], csT16)
                # cs-flat (head-major) into rhs33 row 32
                nc.sync.dma_start(
                    out=_flat_rows(rhs33[32:33, :], NHEADS, Q), in_=csT16)
                # exp(cs) rows -> flat row0 -> partition-broadcast [64, 4096]
                ecs = ckp.tile([NHEADS, Q], F16, tag="ecs")
                nc.scalar.activation(ecs, csl, AF.Exp)
                ecsrep = rpp.tile([D_STATE, NHEADS * Q], F16, tag="ecsrep")
                nc.sync.dma_start(out=_flat_rows(ecsrep[0:1, :], NHEADS, Q),
                                  in_=ecs)
                nc.sync.dma_start(
                    out=ecsrep[1:D_STATE, :],
                    in_=_part_bcast(ecsrep[0:1, :], D_STATE - 1))
                if c == 0:
                    ecsgrep = ecsrep
                else:
                    ecsg = ckp.tile([NHEADS, Q], F16, tag="ecsg")
                    nc.scalar.activation(ecsg, csTg[:, W], AF.Exp)
                    ecsgrep = rpp.tile([D_STATE, NHEADS * Q], F16,
                                       tag="ecsgrep")
                    nc.sync.dma_start(
                        out=_flat_rows(ecsgrep[0:1, :], NHEADS, Q), in_=ecsg)
                    nc.sync.dma_start(
                        out=ecsgrep[1:D_STATE, :],
                        in_=_part_bcast(ecsgrep[0:1, :], D_STATE - 1))
                wT = ckp.tile([NHEADS, Q], F32, tag="wT")
                nc.scalar.activation(wT, csl, AF.Exp, bias=csl[:, Q - 1:Q],
                                     scale=-1.0)
                # [dt | w | cs] -> token-partition layout
                pst = trp.tile([128, 128], F32, tag="pst32")
                i32 = id32[0:32, 0:32]
                nc.tensor.transpose(pst[:, 0:32],
                                    dtT[:, 3 + c * Q:3 + c * Q + Q], i32)
                nc.tensor.transpose(pst[:, 32:64], wT, i32)
                nc.tensor.transpose(pst[0:1, 96:128], csl[:, Q - 1:Q], i32)
                dtwcs = ckp.tile([128, 64], F32, tag="dtwcs")
                nc.scalar.copy(dtwcs, pst[:, 0:64])
                # decay row exp(cs_last) as [1,32], then replicate over 64 parts
                dF = ckp.tile([1, NHEADS], F16, tag="dF")
                nc.scalar.activation(dF, pst[0:1, 96:128], AF.Exp)
                drps = ypsp.tile([128, 128], F32, tag="yps")
                nc.tensor.matmul(drps[0:D_STATE, 0:NHEADS],
                                 ones16[0:1, 0:D_STATE], dF,
                                 start=True, stop=True)
                dRep = ckp.tile([D_STATE, NHEADS], F16, tag="dRep")
                nc.scalar.copy(dRep, drps[0:D_STATE, 0:NHEADS])
                # G^T masked (shared over heads)
                gps = trp.tile([128, 128], F32, tag="pst32")
                nc.tensor.matmul(gps, bt64[:, W], ct64[:, W],
                                 start=True, stop=True)
                gm = ckp.tile([128, 128], F16, tag="gm")
                nc.vector.scalar_tensor_tensor(gm, gps, 1.0, triu16,
                                               op0=AX.mult, op1=AX.mult)
                # B in token-partition layout
                pbt = xdtp.tile([128, 512], F16, tag="pxt16")
                nc.tensor.transpose(pbt[:, 0:64], bt64[:, W], id16[0:64, 0:64])
                btok = ckp.tile([128, 64], F16, tag="btok")
                nc.scalar.copy(btok, pbt[:, 0:64])
                # X -> token-partition; fold dt (xdt) and w (xdtw)
                xdt = ckp.tile([128, D_INNER], F16, tag="xdt")
                xdtw = ckp.tile([128, D_INNER], F16, tag="xdtw")
                for q4 in range(4):
                    pxt = xdtp.tile([128, 512], F16, tag="pxt16")
                    for j in range(4):
                        ti = q4 * 4 + j
                        nc.tensor.transpose(pxt[:, j * 128:(j + 1) * 128],
                                            xcv[ti][:, W], id16)
                    o = q4 * 512
                    nc.vector.tensor_tensor(
                        out=xdt[:, o:o + 512], in0=pxt,
                        in1=_rep_inner(dtwcs[:, 8 * q4:8 * q4 + 8], 64),
                        op=AX.mult)
                    nc.vector.tensor_tensor(
                        out=xdtw[:, o:o + 512], in0=xdt[:, o:o + 512],
                        in1=_rep_inner(dtwcs[:, 32 + 8 * q4:40 + 8 * q4], 64),
                        op=AX.mult)
                # state contribution + h chain
                hcur = hpool.tile([D_STATE, D_INNER], F16, tag="h")
                for q in range(4):
                    o = q * 512
                    sps = spsp.tile([D_STATE, 512], F32, tag="sps")
                    nc.tensor.matmul(sps, btok, xdtw[:, o:o + 512],
                                     start=True, stop=True)
                    if c == 0:
                        nc.scalar.copy(hcur[:, o:o + 512], sps)
                    else:
                        ht = ckp.tile([D_STATE, 512], F16, tag="ht")
                        nc.vector.tensor_tensor(
                            out=ht, in0=hprev[:, o:o + 512],
                            in1=_rep_inner(dRep[:, q * 8:q * 8 + 8], 64),
                            op=AX.mult)
                        nc.vector.tensor_tensor(out=hcur[:, o:o + 512], in0=ht,
                                                in1=sps, op=AX.add)
                # per 2-group (8 heads / 1024 cols) decay-matrix build on PE
                for g2 in range(4):
                    o2 = g2 * 1024
                    dbps = dbp.tile([128, 1024], F32, tag="dbps")
                    nc.tensor.matmul(dbps[:, 0:512], stat33,
                                     rhs33[:, o2:o2 + 512],
                                     start=True, stop=True)
                    nc.tensor.matmul(dbps[:, 512:1024], stat33,
                                     rhs33[:, o2 + 512:o2 + 1024],
                                     start=True, stop=True)
                    # exp(min(x,0)) = exp(-relu(-x)) — keeps clamp on ScalarE
                    nbuf = ebp.tile([128, 1024], F16, tag="nbuf")
                    nc.scalar.activation(nbuf, dbps, AF.Relu, scale=-1.0)
                    ebuf = ebp.tile([128, 1024], F16, tag="ebuf")
                    nc.scalar.activation(ebuf, nbuf, AF.Exp, scale=-1.0)
                    mbuf = ebp.tile([128, 1024], F16, tag="mbuf")
                    nc.vector.tensor_tensor(out=mbuf, in0=ebuf,
                                            in1=_rep_outer(gm, 8), op=AX.mult)
                    ctil = ebp.tile([D_STATE, 1024], F16, tag="ctil")
                    nc.vector.tensor_tensor(
                        out=ctil, in0=ecsrep[:, o2:o2 + 1024],
                        in1=_rep_outer(ct64[:, W], 8), op=AX.mult)
                    if c == 0:
                        ctg = ctil
                    else:
                        ctg = ebp.tile([D_STATE, 1024], F16, tag="ctg")
                        nc.vector.tensor_tensor(
                            out=ctg, in0=ecsgrep[:, o2:o2 + 1024],
                            in1=_rep_outer(ct64[:, W], 8), op=AX.mult)
                    go = (c * 8 + 2 * g2) * 512
                    nc.sync.dma_start(out=ctilg_d[:, go:go + 1024], in_=ctg)
                    # Y for the 8 heads, two per PSUM tile (col-split)
                    for jj in range(4):
                        yps2 = ypsp.tile([128, 128], F32, tag="yps")
                        for par in range(2):
                            j = 2 * jj + par
                            h = 8 * g2 + j
                            tp = None if par == 0 else (0, 64)
                            orow = slice(par * 64, par * 64 + 64)
                            nc.tensor.matmul(
                                yps2[orow, :], xdt[:, h * 64:(h + 1) * 64],
                                mbuf[:, j * 128:(j + 1) * 128],
                                start=True, stop=(c == 0), tile_position=tp)
                            if c > 0:
                                nc.tensor.matmul(
                                    yps2[orow, :],
                                    hprev[:, h * 64:(h + 1) * 64],
                                    ctil[:, j * 128:(j + 1) * 128],
                                    start=False, stop=True, tile_position=tp)
                        ti = (8 * g2 + 2 * jj) // 2
                        nc.vector.scalar_tensor_tensor(
                            ysb[ti][:, W], xcv[ti][:, W], D_sb[:, ti:ti + 1],
                            yps2, op0=AX.mult, op1=AX.add)
                hprev = hcur
            nc.vector.tensor_copy(hfin, hprev)

        if _PH < 3:
            osb0 = perm.tile([128, S], F32)
            nc.scalar.copy(osb0[0:64, :], hfin[0:64, 0:S])
            for dm in range(8):
                nc.sync.dma_start(out=outT[dm * 128:(dm + 1) * 128, :],
                                  in_=osb0)
            return
        # ====== collective: state handoff (AllGather) =======================
        nc.sync.dma_start(out=ccin[:, :], in_=hfin)
        nc.gpsimd.collective_compute(
            "AllGather", AX.bypass,
            replica_groups=[[0, 1, 2, 3, 4, 5, 6, 7]],
            ins=[ccin.ap().opt()], outs=[ccout.ap().opt()])
        with tc.tile_pool(name="hrecp", bufs=2) as hrp:
            # s==1 cores select their pair-partner's block (even-core blocks)
            for r in range(4):
                hblk = hrp.tile([D_STATE, D_INNER], F16, tag="hblk")
                nc.sync.dma_start(out=hblk, in_=ccout[2 * r, :, :])
                if r == 0:
                    nc.vector.tensor_scalar_mul(hinit, hblk, msel_sb[:, 0:1])
                else:
                    nc.vector.scalar_tensor_tensor(
                        hinit, hblk, msel_sb[:, r:r + 1], hinit,
                        op0=AX.mult, op1=AX.add)

        # ================= P3: Delta-Y correction =================
        with tc.tile_pool(name="dyp", bufs=2, space="PSUM") as dypp, \
             tc.tile_pool(name="ctgs", bufs=3) as ctgsp, \
             tc.tile_pool(name="dys", bufs=3) as dysp:
            for g in range(8):
                ctgl = ctgsp.tile([D_STATE, NCHUNK * 512], F16, tag="ctgl")
                for c in range(NCHUNK):
                    go = (c * 8 + g) * 512
                    nc.gpsimd.dma_start(out=ctgl[:, c * 512:(c + 1) * 512],
                                        in_=ctilg_d[:, go:go + 512])
                for hh in range(2):
                    dyps = dypp.tile([128, S], F32, tag="dyps")
                    for par in range(2):
                        j = 2 * hh + par
                        h = 4 * g + j
                        tp = None if par == 0 else (0, 64)
                        orow = slice(par * 64, par * 64 + 64)
                        for c in range(NCHUNK):
                            nc.tensor.matmul(
                                dyps[orow, c * 128:(c + 1) * 128],
                                hinit[:, h * 64:(h + 1) * 64],
                                ctgl[:, c * 512 + j * 128:c * 512 + j * 128 + 128],
                                start=True, stop=True, tile_position=tp)
                    dy16 = dysp.tile([128, S], F16, tag="dy16")
                    nc.scalar.copy(dy16, dyps)
                    ti = (4 * g + 2 * hh) // 2
                    nc.vector.tensor_tensor(out=ysb[ti], in0=ysb[ti], in1=dy16,
                                            op=AX.add)

        # ================= P4: gating, RMSNorm, out_proj =================
        with tc.tile_pool(name="wo", bufs=1) as wop, \
             tc.tile_pool(name="gp", bufs=2) as gp, \
             tc.tile_pool(name="gs", bufs=1) as gs, \
             tc.tile_pool(name="gpsum", bufs=1, space="PSUM") as gpp, \
             tc.tile_pool(name="opsum", bufs=2, space="PSUM") as opp, \
             tc.tile_pool(name="outp", bufs=2) as outp:
            Wo_sb = [wop.tile([128, D_MODEL], F16, tag=f"wo{t}", name=f"wo{t}")
                     for t in range(16)]
            for t in range(16):
                nc.sync.dma_start(out=Wo_sb[t], in_=Wout[t * 128:(t + 1) * 128, :])
            # yg -> xcv tiles (dead after P2); yn -> ysb tiles (in place)
            ssqps = gpp.tile([128, S], F32, tag="ssq")
            for t in range(16):
                nc.vector.tensor_tensor(out=xcv[t], in0=ysb[t], in1=zs[t],
                                        op=AX.mult)
                y2 = gp.tile([128, S], F16, tag="y2")
                nc.vector.tensor_tensor(out=y2, in0=xcv[t], in1=xcv[t],
                                        op=AX.mult)
                nc.tensor.matmul(ssqps[:, 0:512], ones16, y2[:, 0:512],
                                 start=(t == 0), stop=(t == 15))
                nc.tensor.matmul(ssqps[:, 512:1024], ones16, y2[:, 512:1024],
                                 start=(t == 0), stop=(t == 15))
            lnms = gs.tile([128, S], F32, tag="lnms")
            nc.scalar.activation(lnms, ssqps, AF.Ln, bias=eps_sb[:, 0:1],
                                 scale=1.0 / D_INNER)
            rstd = gs.tile([128, S], F16, tag="rstd")
            nc.scalar.activation(rstd, lnms, AF.Exp, scale=-0.5)
            for t in range(16):
                nc.vector.scalar_tensor_tensor(ysb[t], xcv[t], nw_sb[:, t:t + 1],
                                               rstd, op0=AX.mult, op1=AX.mult)
            for dm in range(8):
                ops = opp.tile([128, S], F32, tag="ops")
                for t in range(16):
                    st, sp = (t == 0), (t == 15)
                    lt = Wo_sb[t][:, dm * 128:(dm + 1) * 128]
                    nc.tensor.matmul(ops[:, 0:512], lt, ysb[t][:, 0:512],
                                     start=st, stop=sp)
                    nc.tensor.matmul(ops[:, 512:1024], lt, ysb[t][:, 512:1024],
                                     start=st, stop=sp)
                osb = outp.tile([128, S], F32, tag="osb")
                nc.scalar.copy(osb, ops)
                nc.sync.dma_start(out=outT[dm * 128:(dm + 1) * 128, :], in_=osb)


def _host_prep(inputs):
    u = np.asarray(inputs["u"], np.float32)
    W_in = np.asarray(inputs["W_in"], np.float32)
    conv_w = np.asarray(inputs["conv_w"], np.float32)
    conv_b = np.asarray(inputs["conv_b"], np.float32)
    dt_bias = np.asarray(inputs["dt_bias"], np.float32)
    A_log = np.asarray(inputs["A_log"], np.float32)
    D = np.asarray(inputs["D"], np.float32)
    norm_w = np.asarray(inputs["norm_w"], np.float32)
    W_out = np.asarray(inputs["W_out"], np.float32)

    # extended in_proj: [z | x | B,C | C,B | dt]
    W_ext = np.concatenate([W_in[:, :4224], W_in[:, 4160:4224],
                            W_in[:, 4096:4160], W_in[:, 4224:4256]], axis=1)
    cw = conv_w[:, 0, :].T.astype(np.float32)        # (2176, 4)
    cw_ext = np.concatenate([cw[:2176], cw[2112:2176], cw[2048:2112]], axis=0)
    cb = conv_b.astype(np.float32).reshape(-1, 1)
    cb_ext = np.concatenate([cb[:2176], cb[2112:2176], cb[2048:2112]], axis=0)

    nghs = np.zeros((33, NHEADS * 128), np.float16)
    nghs[0:32, :] = -np.kron(np.eye(NHEADS, dtype=np.float16),
                             np.ones((1, 128), np.float16))

    shared = dict(
        Win=W_ext.astype(np.float16),
        Wout=W_out.astype(np.float16),
        convw=np.ascontiguousarray(cw_ext),
        convb=np.ascontiguousarray(cb_ext),
        dtb=dt_bias.reshape(NHEADS, 1),
        Acol=(-np.exp(A_log)).reshape(NHEADS, 1),
        Dexp=np.repeat(D, HEADDIM).reshape(D_INNER, 1).astype(np.float32),
        nwexp=norm_w.reshape(D_INNER, 1),
        ones_c=np.ones((128, 128), np.float16),
        id16_c=np.eye(128, dtype=np.float16),
        id32_c=np.eye(128, dtype=np.float32),
        triu_c=np.triu(np.ones((128, 128), np.float16)),  # [j,i]=1 if i>=j
        epsc=np.full((128, 1), 1e-5, np.float32),
        nghs_c=nghs,
    )
    in_maps = []
    for core in range(NCORES):
        b, s = core // 2, core % 2
        u_pad = np.zeros((T, D_MODEL), np.float16)
        if s == 0:
            u_pad[3:3 + S] = u[b, :S].astype(np.float16)
        else:
            u_pad[0:3] = u[b, S - 3:S].astype(np.float16)
            u_pad[3:3 + S] = u[b, S:].astype(np.float16)
        m = dict(shared)
        m["uT16"] = np.ascontiguousarray(u_pad.T)
        msel = np.zeros((D_STATE, 4), np.float32)
        if s == 1:
            msel[:, b] = 1.0   # partner is even core 2b -> candidate b
        m["mselp"] = msel
        in_maps.append(m)
    return in_maps


def kernel(**inputs):
    if "nc" not in _CACHE:
        _CACHE["nc"] = build_program()
    nc = _CACHE["nc"]
    in_maps = _host_prep(inputs)
    res = run_bass_kernel_spmd(nc, in_maps, list(range(NCORES)))
    out = np.zeros((B_SZ, L, D_MODEL), np.float32)
    for core in range(NCORES):
        b, s = core // 2, core % 2
        out[b, s * S:(s + 1) * S, :] = res.results[core]["outT"].T
    return out


# revision 28
# speedup vs baseline: 1.3610x; 1.3610x over previous
"""Bass/Trainium2 kernel for nn_BiMambaBlock (Mamba2 block), 8-core SPMD.

Sharding: 8 shards = (batch b in 0..3) x (sequence half s in 0,1); each core
processes 1024 tokens of one batch end-to-end in fp16 on-chip compute
(fp32 accumulation), using the chunked-SSD formulation (chunk Q=128).
Cross-core dependency: SSM state at the sequence midpoint, passed via a
pairwise AllGather; the receiving core applies a low-rank correction
(Delta-Y = C~global @ h_init) before gating/norm/out_proj.

Decay matrices (exp(cs_j - cs_i) per head) are built on the PE via a
rank-33 matmul (ones/cs stationary against a [cs-flat; -head-selector]
moving matrix); partition-broadcasts of exp(cs) rows come from stride-0
DMAs, keeping the vector engine to one fused multiply per tile.
"""
import sys
import numpy as np

for p in ("/opt/trn_rl_repo", "/opt/trn_rl_repo/concourse"):
    if p not in sys.path:
        sys.path.insert(0, p)

import concourse.bass as bass
import concourse.bacc as bacc
import concourse.tile as tile
from concourse import mybir
from concourse.bass_utils import run_bass_kernel_spmd

F32 = mybir.dt.float32
F16 = mybir.dt.float16
AX = mybir.AluOpType
AF = mybir.ActivationFunctionType

D_MODEL = 1024
D_STATE = 64
NHEADS = 32
HEADDIM = 64
D_INNER = 2048
B_SZ = 4
L = 2048
S = 1024          # per-core segment
T = 1152          # padded token window (3 halo + 1024 + 125 pad)
Q = 128           # SSD chunk
NCHUNK = S // Q
NCORES = 8
DIP_EXT = 4384    # z(2048) + x(2048) + [B|C](128) + [C|B](128) + dt(32)

_CACHE = {}


def _rep_inner(sl, rep):
    """[P, n] slice -> [P, n, rep(stride0)] broadcast AP."""
    return bass.AP(tensor=sl.tensor, offset=sl.offset,
                   ap=[sl.ap[0], sl.ap[1], [0, rep]])


def _rep_outer(sl, rep):
    """[P, n] slice -> [P, rep(stride0), n] broadcast AP."""
    return bass.AP(tensor=sl.tensor, offset=sl.offset,
                   ap=[sl.ap[0], [0, rep], sl.ap[1]])


def _col_bcast(col, n):
    """[P, 1] slice -> [P, n(stride0)] broadcast AP."""
    return bass.AP(tensor=col.tensor, offset=col.offset,
                   ap=[col.ap[0], [0, n]])


def _part_bcast(sl, nparts):
    """[1, n] slice -> [1, nparts(stride0), n] broadcast AP (DMA source).

    Partition dim must have nonzero step, so the repeat lives in a
    stride-0 free dim on the single source partition.
    """
    return bass.AP(tensor=sl.tensor, offset=sl.offset,
                   ap=[[1, 1], [0, nparts]] + list(sl.ap[1:]))


def _flat_rows(sl, nrows, ncols):
    """[nrows, ncols] tile viewed as [1, nrows, ncols] free dims (DMA)."""
    # destination AP for a row-major flatten of a multi-partition tile
    return bass.AP(tensor=sl.tensor, offset=sl.offset,
                   ap=[sl.ap[0], [ncols, nrows], [1, ncols]])


def build_program():
    import os
    _PH = int(os.environ.get("KPHASES", "4"))
    _REP = int(os.environ.get("KREPEAT", "1"))
    nc = bacc.Bacc("TRN2", target_bir_lowering=False, debug=False,
                   num_devices=NCORES)
    nc.cache_partition_id()
    _build_body(nc, _PH, _REP)
    nc.compile()
    return nc


def _build_body(nc, _PH, _REP=1):
    uT16 = nc.declare_dram_parameter("uT16", [D_MODEL, T], F16, isOutput=False)
    Win = nc.declare_dram_parameter("Win", [D_MODEL, DIP_EXT], F16, isOutput=False)
    Wout = nc.declare_dram_parameter("Wout", [D_INNER, D_MODEL], F16, isOutput=False)
    convw = nc.declare_dram_parameter("convw", [18 * 128, 4], F32, isOutput=False)
    convb = nc.declare_dram_parameter("convb", [18 * 128, 1], F32, isOutput=False)
    dtb = nc.declare_dram_parameter("dtb", [NHEADS, 1], F32, isOutput=False)
    Acol = nc.declare_dram_parameter("Acol", [NHEADS, 1], F32, isOutput=False)
    Dexp = nc.declare_dram_parameter("Dexp", [D_INNER, 1], F32, isOutput=False)
    nwexp = nc.declare_dram_parameter("nwexp", [D_INNER, 1], F32, isOutput=False)
    ones_c = nc.declare_dram_parameter("ones_c", [128, 128], F16, isOutput=False)
    id16_c = nc.declare_dram_parameter("id16_c", [128, 128], F16, isOutput=False)
    id32_c = nc.declare_dram_parameter("id32_c", [128, 128], F32, isOutput=False)
    triu_c = nc.declare_dram_parameter("triu_c", [128, 128], F16, isOutput=False)
    nghs_c = nc.declare_dram_parameter("nghs_c", [33, NHEADS * 128], F16,
                                       isOutput=False)
    mselp = nc.declare_dram_parameter("mselp", [D_STATE, 4], F32, isOutput=False)
    epsc = nc.declare_dram_parameter("epsc", [128, 1], F32, isOutput=False)
    outT = nc.declare_dram_parameter("outT", [D_MODEL, S], F32, isOutput=True)

    ctilg_d = nc.dram_tensor("ctilg_d", [D_STATE, NCHUNK * NHEADS * 128], F16)
    ccin = nc.dram_tensor("ccin", [D_STATE, D_INNER], F16)
    ccout = nc.dram_tensor("ccout", [NCORES, D_STATE, D_INNER], F16,
                           addr_space="Shared")

    for _rep in range(_REP):
        _build_once(nc, _PH, uT16, Win, Wout, convw, convb, dtb, Acol, Dexp,
                    nwexp, ones_c, id16_c, id32_c, triu_c, mselp, epsc, nghs_c,
                    outT, ctilg_d, ccin, ccout)


def _build_once(nc, _PH, uT16, Win, Wout, convw, convb, dtb, Acol, Dexp,
                nwexp, ones_c, id16_c, id32_c, triu_c, mselp, epsc, nghs_c,
                outT, ctilg_d, ccin, ccout):
    import os
    from contextlib import ExitStack
    with tile.TileContext(nc) as tc, ExitStack() as ctx:
        perm = ctx.enter_context(tc.tile_pool(name="perm", bufs=1))

        ones16 = perm.tile([128, 128], F16)
        nc.sync.dma_start(out=ones16, in_=ones_c[:, :])
        id16 = perm.tile([128, 128], F16)
        nc.sync.dma_start(out=id16, in_=id16_c[:, :])
        id32 = perm.tile([128, 128], F32)
        nc.sync.dma_start(out=id32, in_=id32_c[:, :])
        triu16 = perm.tile([128, 128], F16)
        nc.sync.dma_start(out=triu16, in_=triu_c[:, :])
        dtb_sb = perm.tile([NHEADS, 1], F32)
        nc.sync.dma_start(out=dtb_sb, in_=dtb[:, :])
        A_sb = perm.tile([NHEADS, 1], F32)
        nc.sync.dma_start(out=A_sb, in_=Acol[:, :])
        msel_sb = perm.tile([D_STATE, 4], F32)
        nc.sync.dma_start(out=msel_sb, in_=mselp[:, :])
        eps_sb = perm.tile([128, 1], F32)
        nc.sync.dma_start(out=eps_sb, in_=epsc[:, :])
        cw_sb = perm.tile([128, 18 * 4], F32)
        cb_sb = perm.tile([128, 18], F32)
        for t in range(18):
            nc.sync.dma_start(out=cw_sb[:, t * 4:(t + 1) * 4],
                              in_=convw[t * 128:(t + 1) * 128, :])
            nc.sync.dma_start(out=cb_sb[:, t:t + 1],
                              in_=convb[t * 128:(t + 1) * 128, :])
        D_sb = perm.tile([128, 16], F32)
        nw_sb = perm.tile([128, 16], F32)
        for t in range(16):
            nc.sync.dma_start(out=D_sb[:, t:t + 1],
                              in_=Dexp[t * 128:(t + 1) * 128, :])
            nc.sync.dma_start(out=nw_sb[:, t:t + 1],
                              in_=nwexp[t * 128:(t + 1) * 128, :])
        # rhs33: rows0:32 = -head-selector, row32 = per-chunk cs-flat (DMA'd)
        rhs33s, stat33s = [], []
        for par in range(2):
            r33 = perm.tile([33, NHEADS * 128], F16, tag=f"rhs33_{par}",
                            name=f"rhs33_{par}")
            nc.sync.dma_start(out=r33, in_=nghs_c[:, :])
            s33 = perm.tile([33, 128], F16, tag=f"stat33_{par}",
                            name=f"stat33_{par}")
            nc.vector.memset(s33[32:33, :], 1.0)
            rhs33s.append(r33)
            stat33s.append(s33)

        zs = [perm.tile([128, S], F16, tag=f"zs{t}", name=f"zs{t}")
              for t in range(16)]
        xcv = [perm.tile([128, S], F16, tag=f"xcv{t}", name=f"xcv{t}")
               for t in range(16)]
        bt64 = perm.tile([D_STATE, S], F16)   # B^T  (state-dim on partitions)
        ct64 = perm.tile([D_STATE, S], F16)   # C^T
        dtT = perm.tile([NHEADS, T], F32)

        # allocated early so its SBUF region does not alias W_in space ->
        # the scan can run while z matmuls still stream
        csTg = perm.tile([NHEADS, S], F32)

        # ================= P1: in_proj + conv =================
        # ct order: dt first (34), then xBC (16..33), then z (0..15) so the
        # scan + P2 chunks can start while z matmuls still run.
        with tc.tile_pool(name="win", bufs=1) as winp, \
             tc.tile_pool(name="ut", bufs=1) as utp, \
             tc.tile_pool(name="ippsum", bufs=2, space="PSUM") as ipp, \
             tc.tile_pool(name="cvtmp", bufs=2) as cvp:
            uT = [utp.tile([128, T], F16, tag=f"ut{k}", name=f"ut{k}")
                  for k in range(8)]
            for k in range(8):
                nc.sync.dma_start(out=uT[k],
                                  in_=uT16[k * 128:(k + 1) * 128, :])
            W_sb = [winp.tile([128, DIP_EXT], F16, tag=f"w{k}", name=f"w{k}")
                    for k in range(8)]
            for k in range(8):
                nc.sync.dma_start(out=W_sb[k], in_=Win[k * 128:(k + 1) * 128, :])

            ct_order = [34] + list(range(16, 34)) + list(range(0, 16))
            _CTMAX = int(os.environ.get('KCT', '35'))
            for ct in ct_order[:_CTMAX]:
                ncols = 128 if ct < 34 else 32
                c0 = ct * 128
                ip = ipp.tile([128, T], F32, tag="ip")
                for k in range(8):
                    st, sp = (k == 0), (k == 7)
                    lt = W_sb[k][:, c0:c0 + ncols]
                    nc.tensor.matmul(ip[0:ncols, 0:512], lt, uT[k][:, 0:512],
                                     start=st, stop=sp)
                    nc.tensor.matmul(ip[0:ncols, 512:1024], lt,
                                     uT[k][:, 512:1024], start=st, stop=sp)
                    nc.tensor.matmul(ip[0:ncols, 1024:1152], lt,
                                     uT[k][:, 1024:1152], start=st, stop=sp)
                if ct < 16:
                    nc.scalar.activation(zs[ct], ip[:, 3:3 + S], AF.Silu)
                elif ct < 34:
                    xi = ct - 16
                    rows = 128 if xi < 16 else 64
                    xbc = cvp.tile([128, T], F16, tag="xbc")
                    nc.scalar.copy(xbc[0:rows, :], ip[0:rows, :])
                    w4 = cw_sb[:, xi * 4:(xi + 1) * 4]
                    t1 = cvp.tile([128, S], F16, tag="cv1")
                    nc.vector.tensor_scalar_mul(t1[0:rows, :], xbc[0:rows, 0:S],
                                                w4[0:rows, 0:1])
                    t2 = cvp.tile([128, S], F16, tag="cv2")
                    nc.vector.scalar_tensor_tensor(
                        t2[0:rows, :], xbc[0:rows, 1:1 + S], w4[0:rows, 1:2],
                        t1[0:rows, :], op0=AX.mult, op1=AX.add)
                    t3 = cvp.tile([128, S], F16, tag="cv1")
                    nc.vector.scalar_tensor_tensor(
                        t3[0:rows, :], xbc[0:rows, 2:2 + S], w4[0:rows, 2:3],
                        t2[0:rows, :], op0=AX.mult, op1=AX.add)
                    t4 = cvp.tile([128, S], F16, tag="cv2")
                    nc.vector.scalar_tensor_tensor(
                        t4[0:rows, :], xbc[0:rows, 3:3 + S], w4[0:rows, 3:4],
                        t3[0:rows, :], op0=AX.mult, op1=AX.add)
                    if xi < 16:
                        nc.scalar.activation(xcv[xi], t4, AF.Silu,
                                             bias=cb_sb[:, xi:xi + 1])
                    elif xi == 16:
                        nc.scalar.activation(bt64, t4[0:64, :], AF.Silu,
                                             bias=cb_sb[0:64, xi:xi + 1])
                    else:
                        nc.scalar.activation(ct64, t4[0:64, :], AF.Silu,
                                             bias=cb_sb[0:64, xi:xi + 1])
                else:
                    # cumulative sum of dt*A over the segment (scan)
                    with tc.tile_pool(name="scanp", bufs=1) as scp:
                        dte = scp.tile([NHEADS, T], F32, tag="dte")
                        nc.scalar.activation(dte, ip[0:NHEADS, :], AF.Exp,
                                             bias=dtb_sb[:, 0:1])
                        nc.scalar.activation(dtT, dte, AF.Ln, bias=1.0)
                        aT = scp.tile([NHEADS, S], F32)
                        zrow = scp.tile([NHEADS, S], F32)
                        nc.vector.tensor_scalar_mul(aT, dtT[:, 3:3 + S],
                                                    A_sb[:, 0:1])
                        nc.vector.memset(zrow, 0.0)
                        nc.vector.tensor_tensor_scan(csTg, aT, zrow, 0.0,
                                                     op0=AX.add, op1=AX.add)

        if _CTMAX < 35 or _PH < 2:
            osb0 = perm.tile([128, S], F32)
            nc.scalar.copy(osb0[0:NHEADS, :], csTg)
            for dm in range(8):
                nc.sync.dma_start(out=outT[dm * 128:(dm + 1) * 128, :],
                                  in_=osb0)
            return

        # pool for tensors first written during/after P2 (reuses W_in space)
        perm2 = ctx.enter_context(tc.tile_pool(name="perm2", bufs=1))
        ysb = [perm2.tile([128, S], F16, tag=f"ysb{t}", name=f"ysb{t}")
               for t in range(16)]
        hfin = perm2.tile([D_STATE, D_INNER], F16)
        hinit = perm2.tile([D_STATE, D_INNER], F16)

        # ================= P2: SSD chunks =================
        with tc.tile_pool(name="trp", bufs=1, space="PSUM") as trp, \
             tc.tile_pool(name="xdtp", bufs=2, space="PSUM") as xdtp, \
             tc.tile_pool(name="dbp", bufs=2, space="PSUM") as dbp, \
             tc.tile_pool(name="spsp", bufs=1, space="PSUM") as spsp, \
             tc.tile_pool(name="ypsp", bufs=2, space="PSUM") as ypsp, \
             tc.tile_pool(name="ck", bufs=2) as ckp, \
             tc.tile_pool(name="eb", bufs=2) as ebp, \
             tc.tile_pool(name="rp", bufs=2) as rpp, \
             tc.tile_pool(name="hp", bufs=2) as hpool:
            hprev = None
            ctils = [None] * 4
            for c in range(NCHUNK):
                W = slice(c * Q, (c + 1) * Q)
                csl = ckp.tile([NHEADS, Q], F32, tag="csl")
                if c == 0:
                    nc.vector.tensor_copy(csl, csTg[:, W])
                else:
                    nc.vector.tensor_scalar(csl, csTg[:, W],
                                            csTg[:, c * Q - 1:c * Q], None,
                                            op0=AX.subtract)
                csT16 = ckp.tile([NHEADS, Q], F16, tag="cst16")
                nc.vector.tensor_copy(csT16, csl)
                # decay-delta matmul stationary: rows0:32=cs, row32=ones
                stat33 = stat33s[c % 2]
                rhs33 = rhs33s[c % 2]
                nc.vector.tensor_copy(stat33[0:32, :], csT16)
                # cs-flat (head-major) into rhs33 row 32
                nc.sync.dma_start(
                    out=_flat_rows(rhs33[32:33, :], NHEADS, Q), in_=csT16)
                # exp(cs) rows -> flat row0 -> partition-broadcast [64, 4096]
                ecs = ckp.tile([NHEADS, Q], F16, tag="ecs")
                nc.scalar.activation(ecs, csl, AF.Exp)
                ecsrep = rpp.tile([D_STATE, NHEADS * Q], F16, tag="ecsrep")
                nc.sync.dma_start(out=_flat_rows(ecsrep[0:1, :], NHEADS, Q),
                                  in_=ecs)
                nc.sync.dma_start(
                    out=ecsrep[1:D_STATE, :],
                    in_=_part_bcast(ecsrep[0:1, :], D_STATE - 1))

                wT = ckp.tile([NHEADS, Q], F32, tag="wT")
                nc.scalar.activation(wT, csl, AF.Exp, bias=csl[:, Q - 1:Q],
                                     scale=-1.0)
                # [dt | w | cs] -> token-partition layout
                pst = trp.tile([128, 128], F32, tag="pst32")
                i32 = id32[0:32, 0:32]
                nc.tensor.transpose(pst[:, 0:32],
                                    dtT[:, 3 + c * Q:3 + c * Q + Q], i32)
                nc.tensor.transpose(pst[:, 32:64], wT, i32)
                nc.tensor.transpose(pst[0:1, 96:128], csl[:, Q - 1:Q], i32)
                scal = None
                if c > 0:
                    nc.tensor.transpose(pst[0:1, 64:96],
                                        csTg[:, c * Q - 1:c * Q], i32)
                    scalr = ckp.tile([1, NHEADS], F16, tag="scalr")
                    nc.scalar.activation(scalr, pst[0:1, 64:96], AF.Exp)
                    scal = ckp.tile([D_STATE, NHEADS], F16, tag="scal")
                    nc.sync.dma_start(
                        out=scal, in_=_part_bcast(scalr[0:1, :], D_STATE))
                dtwcs = ckp.tile([128, 64], F32, tag="dtwcs")
                nc.scalar.copy(dtwcs, pst[:, 0:64])
                # decay row exp(cs_last) as [1,32], then replicate over 64 parts
                dF = ckp.tile([1, NHEADS], F16, tag="dF")
                nc.scalar.activation(dF, pst[0:1, 96:128], AF.Exp)
                drps = ypsp.tile([128, 128], F32, tag="yps")
                nc.tensor.matmul(drps[0:D_STATE, 0:NHEADS],
                                 ones16[0:1, 0:D_STATE], dF,
                                 start=True, stop=True)
                dRep = ckp.tile([D_STATE, NHEADS], F16, tag="dRep")
                nc.scalar.copy(dRep, drps[0:D_STATE, 0:NHEADS])
                # G^T masked (shared over heads)
                gps = trp.tile([128, 128], F32, tag="pst32")
                nc.tensor.matmul(gps, bt64[:, W], ct64[:, W],
                                 start=True, stop=True)
                gm = ckp.tile([128, 128], F16, tag="gm")
                nc.vector.scalar_tensor_tensor(gm, gps, 1.0, triu16,
                                               op0=AX.mult, op1=AX.mult)
                # B in token-partition layout
                pbt = xdtp.tile([128, 512], F16, tag="pxt16")
                nc.tensor.transpose(pbt[:, 0:64], bt64[:, W], id16[0:64, 0:64])
                btok = ckp.tile([128, 64], F16, tag="btok")
                nc.scalar.copy(btok, pbt[:, 0:64])
                # X -> token-partition; fold dt (xdt) and w (xdtw);
                # state update interleaved per 512-column block
                xdt = ckp.tile([128, D_INNER], F16, tag="xdt")
                hcur = hpool.tile([D_STATE, D_INNER], F16, tag="h")
                for q4 in range(4):
                    pxt = xdtp.tile([128, 512], F16, tag="pxt16")
                    for j in range(4):
                        ti = q4 * 4 + j
                        nc.tensor.transpose(pxt[:, j * 128:(j + 1) * 128],
                                            xcv[ti][:, W], id16)
                    o = q4 * 512
                    nc.vector.tensor_tensor(
                        out=xdt[:, o:o + 512], in0=pxt,
                        in1=_rep_inner(dtwcs[:, 8 * q4:8 * q4 + 8], 64),
                        op=AX.mult)
                    xdtw = ckp.tile([128, 512], F16, tag="xdtw")
                    nc.vector.tensor_tensor(
                        out=xdtw, in0=xdt[:, o:o + 512],
                        in1=_rep_inner(dtwcs[:, 32 + 8 * q4:40 + 8 * q4], 64),
                        op=AX.mult)
                    sps = spsp.tile([D_STATE, 512], F32, tag="sps")
                    nc.tensor.matmul(sps, btok, xdtw,
                                     start=True, stop=True)
                    if c == 0:
                        nc.scalar.copy(hcur[:, o:o + 512], sps)
                    else:
                        ht = ckp.tile([D_STATE, 512], F16, tag="ht")
                        nc.vector.tensor_tensor(
                            out=ht, in0=hprev[:, o:o + 512],
                            in1=_rep_inner(dRep[:, q4 * 8:q4 * 8 + 8], 64),
                            op=AX.mult)
                        nc.vector.tensor_tensor(out=hcur[:, o:o + 512], in0=ht,
                                                in1=sps, op=AX.add)
                # ctil/ctg per 8-head block (from partition-broadcast rows)
                for g2 in range(4):
                    o2 = g2 * 1024
                    ctil = ebp.tile([D_STATE, 1024], F16, tag=f"ctil{g2}")
                    nc.vector.tensor_tensor(
                        out=ctil, in0=ecsrep[:, o2:o2 + 1024],
                        in1=_rep_outer(ct64[:, W], 8), op=AX.mult)
                    if c == 0:
                        ctg = ctil
                    else:
                        ctg = ebp.tile([D_STATE, 1024], F16, tag="ctg")
                        nc.vector.tensor_tensor(
                            out=ctg, in0=ctil,
                            in1=_rep_inner(scal[:, 8 * g2:8 * g2 + 8], Q),
                            op=AX.mult)
                    go = (c * 8 + 2 * g2) * 512
                    nc.sync.dma_start(out=ctilg_d[:, go:go + 1024], in_=ctg)
                    ctils[g2] = ctil
                # per 4-head group: decay matrix on PE, exp-clamp on ScalarE
                for g in range(8):
                    o4 = g * 512
                    dbps = dbp.tile([128, 512], F32, tag="dbps")
                    nc.tensor.matmul(dbps, stat33, rhs33[:, o4:o4 + 512],
                                     start=True, stop=True)
                    # exp(min(x,0)) = exp(-relu(-x)) — keeps clamp on ScalarE
                    nbuf = ebp.tile([128, 512], F16, tag="nbuf")
                    nc.scalar.activation(nbuf, dbps, AF.Relu, scale=-1.0)
                    ebuf = ebp.tile([128, 512], F16, tag="ebuf")
                    nc.scalar.activation(ebuf, nbuf, AF.Exp, scale=-1.0)
                    mbuf = ebp.tile([128, 512], F16, tag="mbuf")
                    nc.vector.tensor_tensor(out=mbuf, in0=ebuf,
                                            in1=_rep_outer(gm, 4), op=AX.mult)
                    ctil = ctils[g // 2]
                    oc = (g % 2) * 512
                    # Y for the 4 heads, two per PSUM tile (col-split)
                    for jj in range(2):
                        yps2 = ypsp.tile([128, 128], F32, tag="yps")
                        for par in range(2):
                            j = 2 * jj + par
                            h = 4 * g + j
                            tp = None if par == 0 else (0, 64)
                            orow = slice(par * 64, par * 64 + 64)
                            nc.tensor.matmul(
                                yps2[orow, :], xdt[:, h * 64:(h + 1) * 64],
                                mbuf[:, j * 128:(j + 1) * 128],
                                start=True, stop=(c == 0), tile_position=tp)
                            if c > 0:
                                nc.tensor.matmul(
                                    yps2[orow, :],
                                    hprev[:, h * 64:(h + 1) * 64],
                                    ctil[:, oc + j * 128:oc + (j + 1) * 128],
                                    start=False, stop=True, tile_position=tp)
                        ti = (4 * g + 2 * jj) // 2
                        nc.vector.scalar_tensor_tensor(
                            ysb[ti][:, W], xcv[ti][:, W], D_sb[:, ti:ti + 1],
                            yps2, op0=AX.mult, op1=AX.add)
                hprev = hcur
            nc.vector.tensor_copy(hfin, hprev)

        if _PH < 3:
            osb0 = perm.tile([128, S], F32)
            nc.scalar.copy(osb0[0:64, :], hfin[0:64, 0:S])
            for dm in range(8):
                nc.sync.dma_start(out=outT[dm * 128:(dm + 1) * 128, :],
                                  in_=osb0)
            return
        # ====== collective: state handoff (AllGather) =======================
        nc.sync.dma_start(out=ccin[:, :], in_=hfin)
        nc.gpsimd.collective_compute(
            "AllGather", AX.bypass,
            replica_groups=[[0, 1, 2, 3, 4, 5, 6, 7]],
            ins=[ccin.ap().opt()], outs=[ccout.ap().opt()])
        with tc.tile_pool(name="hrecp", bufs=2) as hrp:
            # s==1 cores select their pair-partner's block (even-core blocks)
            for r in range(4):
                hblk = hrp.tile([D_STATE, D_INNER], F16, tag="hblk")
                nc.sync.dma_start(out=hblk, in_=ccout[2 * r, :, :])
                if r == 0:
                    nc.vector.tensor_scalar_mul(hinit, hblk, msel_sb[:, 0:1])
                else:
                    nc.vector.scalar_tensor_tensor(
                        hinit, hblk, msel_sb[:, r:r + 1], hinit,
                        op0=AX.mult, op1=AX.add)

        # ================= P3: Delta-Y correction =================
        with tc.tile_pool(name="dyp", bufs=2, space="PSUM") as dypp, \
             tc.tile_pool(name="ctgs", bufs=3) as ctgsp, \
             tc.tile_pool(name="dys", bufs=3) as dysp:
            for g in range(8):
                ctgl = ctgsp.tile([D_STATE, NCHUNK * 512], F16, tag="ctgl")
                for c in range(NCHUNK):
                    go = (c * 8 + g) * 512
                    nc.gpsimd.dma_start(out=ctgl[:, c * 512:(c + 1) * 512],
                                        in_=ctilg_d[:, go:go + 512])
                for hh in range(2):
                    dyps = dypp.tile([128, S], F32, tag="dyps")
                    for par in range(2):
                        j = 2 * hh + par
                        h = 4 * g + j
                        tp = None if par == 0 else (0, 64)
                        orow = slice(par * 64, par * 64 + 64)
                        for c in range(NCHUNK):
                            nc.tensor.matmul(
                                dyps[orow, c * 128:(c + 1) * 128],
                                hinit[:, h * 64:(h + 1) * 64],
                                ctgl[:, c * 512 + j * 128:c * 512 + j * 128 + 128],
                                start=True, stop=True, tile_position=tp)
                    dy16 = dysp.tile([128, S], F16, tag="dy16")
                    nc.scalar.copy(dy16, dyps)
                    ti = (4 * g + 2 * hh) // 2
                    nc.vector.tensor_tensor(out=ysb[ti], in0=ysb[ti], in1=dy16,
                                            op=AX.add)

        # ================= P4: gating, RMSNorm, out_proj =================
        with tc.tile_pool(name="wo", bufs=1) as wop, \
             tc.tile_pool(name="gp", bufs=2) as gp, \
             tc.tile_pool(name="gs", bufs=1) as gs, \
             tc.tile_pool(name="gpsum", bufs=1, space="PSUM") as gpp, \
             tc.tile_pool(name="opsum", bufs=2, space="PSUM") as opp, \
             tc.tile_pool(name="outp", bufs=2) as outp:
            Wo_sb = [wop.tile([128, D_MODEL], F16, tag=f"wo{t}", name=f"wo{t}")
                     for t in range(16)]
            for t in range(16):
                nc.sync.dma_start(out=Wo_sb[t], in_=Wout[t * 128:(t + 1) * 128, :])
            # yg -> xcv tiles (dead after P2); yn -> ysb tiles (in place)
            ssqps = gpp.tile([128, S], F32, tag="ssq")
            for t in range(16):
                nc.vector.tensor_tensor(out=xcv[t], in0=ysb[t], in1=zs[t],
                                        op=AX.mult)
                y2 = gp.tile([128, S], F16, tag="y2")
                nc.vector.tensor_tensor(out=y2, in0=xcv[t], in1=xcv[t],
                                        op=AX.mult)
                nc.tensor.matmul(ssqps[:, 0:512], ones16, y2[:, 0:512],
                                 start=(t == 0), stop=(t == 15))
                nc.tensor.matmul(ssqps[:, 512:1024], ones16, y2[:, 512:1024],
                                 start=(t == 0), stop=(t == 15))
            lnms = gs.tile([128, S], F32, tag="lnms")
            nc.scalar.activation(lnms, ssqps, AF.Ln, bias=eps_sb[:, 0:1],
                                 scale=1.0 / D_INNER)
            rstd = gs.tile([128, S], F16, tag="rstd")
            nc.scalar.activation(rstd, lnms, AF.Exp, scale=-0.5)
            for t in range(16):
                nc.vector.scalar_tensor_tensor(ysb[t], xcv[t], nw_sb[:, t:t + 1],
                                               rstd, op0=AX.mult, op1=AX.mult)
            for dm in range(8):
                ops = opp.tile([128, S], F32, tag="ops")
                for t in range(16):
                    st, sp = (t == 0), (t == 15)
                    lt = Wo_sb[t][:, dm * 128:(dm + 1) * 128]
                    nc.tensor.matmul(ops[:, 0:512], lt, ysb[t][:, 0:512],
                                     start=st, stop=sp)
                    nc.tensor.matmul(ops[:, 512:1024], lt, ysb[t][:, 512:1024],
                                     start=st, stop=sp)
                osb = outp.tile([128, S], F32, tag="osb")
                nc.scalar.copy(osb, ops)
                nc.sync.dma_start(out=outT[dm * 128:(dm + 1) * 128, :], in_=osb)


def _host_prep(inputs):
    u = np.asarray(inputs["u"], np.float32)
    W_in = np.asarray(inputs["W_in"], np.float32)
    conv_w = np.asarray(inputs["conv_w"], np.float32)
    conv_b = np.asarray(inputs["conv_b"], np.float32)
    dt_bias = np.asarray(inputs["dt_bias"], np.float32)
    A_log = np.asarray(inputs["A_log"], np.float32)
    D = np.asarray(inputs["D"], np.float32)
    norm_w = np.asarray(inputs["norm_w"], np.float32)
    W_out = np.asarray(inputs["W_out"], np.float32)

    # extended in_proj: [z | x | B,C | C,B | dt]
    W_ext = np.concatenate([W_in[:, :4224], W_in[:, 4160:4224],
                            W_in[:, 4096:4160], W_in[:, 4224:4256]], axis=1)
    cw = conv_w[:, 0, :].T.astype(np.float32)        # (2176, 4)
    cw_ext = np.concatenate([cw[:2176], cw[2112:2176], cw[2048:2112]], axis=0)
    cb = conv_b.astype(np.float32).reshape(-1, 1)
    cb_ext = np.concatenate([cb[:2176], cb[2112:2176], cb[2048:2112]], axis=0)

    nghs = np.zeros((33, NHEADS * 128), np.float16)
    nghs[0:32, :] = -np.kron(np.eye(NHEADS, dtype=np.float16),
                             np.ones((1, 128), np.float16))

    shared = dict(
        Win=W_ext.astype(np.float16),
        Wout=W_out.astype(np.float16),
        convw=np.ascontiguousarray(cw_ext),
        convb=np.ascontiguousarray(cb_ext),
        dtb=dt_bias.reshape(NHEADS, 1),
        Acol=(-np.exp(A_log)).reshape(NHEADS, 1),
        Dexp=np.repeat(D, HEADDIM).reshape(D_INNER, 1).astype(np.float32),
        nwexp=norm_w.reshape(D_INNER, 1),
        ones_c=np.ones((128, 128), np.float16),
        id16_c=np.eye(128, dtype=np.float16),
        id32_c=np.eye(128, dtype=np.float32),
        triu_c=np.triu(np.ones((128, 128), np.float16)),  # [j,i]=1 if i>=j
        epsc=np.full((128, 1), 1e-5, np.float32),
        nghs_c=nghs,
    )
    in_maps = []
    for core in range(NCORES):
        b, s = core // 2, core % 2
        u_pad = np.zeros((T, D_MODEL), np.float16)
        if s == 0:
            u_pad[3:3 + S] = u[b, :S].astype(np.float16)
        else:
            u_pad[0:3] = u[b, S - 3:S].astype(np.float16)
            u_pad[3:3 + S] = u[b, S:].astype(np.float16)
        m = dict(shared)
        m["uT16"] = np.ascontiguousarray(u_pad.T)
        msel = np.zeros((D_STATE, 4), np.float32)
        if s == 1:
            msel[:, b] = 1.0   # partner is even core 2b -> candidate b
        m["mselp"] = msel
        in_maps.append(m)
    return in_maps


def kernel(**inputs):
    if "nc" not in _CACHE:
        _CACHE["nc"] = build_program()
    nc = _CACHE["nc"]
    in_maps = _host_prep(inputs)
    res = run_bass_kernel_spmd(nc, in_maps, list(range(NCORES)))
    out = np.zeros((B_SZ, L, D_MODEL), np.float32)
    for core in range(NCORES):
        b, s = core // 2, core % 2
        out[b, s * S:(s + 1) * S, :] = res.results[core]["outT"].T
    return out
